# revision 8
# baseline (speedup 1.0000x reference)
"""2-layer GAT on 8 NeuronCores (Trainium2, Bass/Tile) — v2.

Strategy: dst-node sharding with a single global record layout shared by
both layers. Nodes are assigned degree-balanced slots within their owning
core's 49 tiles of 128; the global row order (gperm) is region-major:
region A = slots 0:3200 of each core (rows c*3200+s, 25600 total), region
B = slots 3200:6272 (rows 25600 + c*3072 + (s-3200)). Both layers use the
SAME edge index tensors (ragged per-tile chunk counts, int16 indices with
a lo/hi split at the A/B boundary).

Records are [z(256, stored feature-major d*4+h) | el(4) | ee-scratch(4)]
in rows of stride 384 fp16 (dma_gather needs 256B-aligned elem/stride).
z stored [d,h]-major makes the alpha broadcast-multiply hit the DVE 2x
perf mode (broadcast lands on a middle dim, last dim stays packed).

Layer-2 records are AllGathered in two pieces (full record rows — the
BIR verifier requires contiguous collective patterns): AG-A (region A)
issued after layer-1 tile 24 so it flies under the remaining layer-1
compute; AG-B at layer-1 end. Layer 2 runs in two passes: pass A
aggregates region-A edges into an SBUF accumulator while AG-B is in
flight; pass B adds region-B edges (PSUM preloaded from the accumulator
via an identity matmul) and finishes the epilogue. Epilogues use
ACT-engine per-head scale pointers (1/s softmax normalization fused into
Relu/Copy) when the biases are zero, keeping the vector engine off the
critical path.
"""
import os
import numpy as np

N = 50000
E = 800000
IN_F, HID, OUT, HEADS = 128, 64, 64, 4
D1 = HEADS * HID   # 256
D2 = HEADS * OUT   # 256
NEG = 0.2
NCORES = 8
SHN = N // NCORES          # 6250 dst nodes per core
TILES = 49
SH = TILES * 128           # 6272 slots per core
NP = NCORES * SH           # 50176 rows
ATILES = 17                # region A tiles per core (B capped at 32768 rows for int16)
ASL = ATILES * 128         # 3200 region-A slots per core
BSL = SH - ASL             # 3072 region-B slots
NA = NCORES * ASL          # 25600 region-A rows (lo/hi split)
NB = NCORES * BSL          # 24576 region-B rows
RECW = 384                 # DRAM record stride (fp16 cols); z 0:256, el 256:260
RECU = 264                 # used cols (260 written; 260:264 SBUF ee scratch)


def _host_prep(x, src, dst, W1, al1, ar1, b1, W2, al2, ar2, b2):
    """Pure-numpy preprocessing: slots, global perm, ragged edge chunking."""
    f32, f16 = np.float32, np.float16
    deg = np.bincount(dst, minlength=N)

    # per-core tile assignment (degree balanced round robin)
    slot_of = np.full(N, -1, np.int64)
    node_of_slot = np.full((NCORES, SH), -1, np.int64)
    for c in range(NCORES):
        nodes = np.arange(c * SHN, (c + 1) * SHN)
        order = nodes[np.argsort(-deg[nodes], kind="stable")]
        i = np.arange(order.size)
        s = (i % TILES) * 128 + i // TILES
        slot_of[order] = s
        node_of_slot[c, s] = order

    # global row (both layers): region-major
    cn = np.arange(N) // SHN
    gperm = np.where(slot_of < ASL,
                     cn * ASL + slot_of,
                     NA + cn * BSL + (slot_of - ASL))
    # node stored at row gperm[n]; xTp columns in that order
    node_of_row = np.zeros(NP, np.int64)   # pad rows read node 0 (unused)
    node_of_row[gperm] = np.arange(N)

    ecore = dst // SHN
    etile = slot_of[dst] // 128
    edstl = slot_of[dst] % 128
    erow = gperm[src]

    per = {}
    for c in range(NCORES):
        sel = np.flatnonzero(ecore == c)
        pt = etile[sel]
        for t in range(TILES):
            m = sel[pt == t]
            per[(c, t)] = (erow[m], edstl[m])

    TLs, THs = [], []
    for t in range(TILES):
        nlo = max(int((per[(c, t)][0] < NA).sum()) for c in range(NCORES))
        nhi = max(int((per[(c, t)][0] >= NA).sum()) for c in range(NCORES))
        TLs.append(-(-nlo // 128))
        THs.append(-(-nhi // 128))
    TLs, THs = tuple(TLs), tuple(THs)
    SL, SHI = sum(TLs), sum(THs)

    def pack_idx(vals, TT):
        padded = np.zeros(TT * 128, np.int16)
        padded[:vals.size] = vals.astype(np.int16)
        ii = np.arange(TT * 128)
        w = np.zeros((16, TT * 8), np.int16)
        w[ii % 16, ii // 16] = padded
        return np.tile(w, (8, 1))

    il = np.zeros((NCORES, 128, SL * 8), np.int16)
    ih = np.zeros((NCORES, 128, SHI * 8), np.int16)
    dstl = np.full((NCORES, 128, SL + SHI), -1.0, np.float32)
    ol = oh_ = od = 0
    for t in range(TILES):
        TL, TH = TLs[t], THs[t]
        for c in range(NCORES):
            rows, dl = per[(c, t)]
            lo = rows < NA
            lv, hv = rows[lo], rows[~lo] - NA
            dlo, dhi = dl[lo], dl[~lo]
            if TL:
                il[c, :, ol * 8:(ol + TL) * 8] = pack_idx(lv, TL)
            if TH:
                ih[c, :, oh_ * 8:(oh_ + TH) * 8] = pack_idx(hv, TH)
            for vals, TT, off in ((dlo, TL, od), (dhi, TH, od + TL)):
                if TT == 0:
                    continue
                dpad = np.full(TT * 128, -1.0, f32)
                dpad[:vals.size] = vals
                ii = np.arange(TT * 128)
                dstl[c, ii % 128, off + ii // 128] = dpad
        ol += TL
        oh_ += TH
        od += TL + TH

    # x transposed, global row order (same for all cores)
    xT = np.ascontiguousarray(x.T).astype(f16)          # [128, N]
    xTp = xT[:, node_of_row]                            # [128, NP]
    # per-core own-shard x (slot order) for er computation
    xoT = np.zeros((NCORES, IN_F, SH), f16)
    for c in range(NCORES):
        valid = node_of_slot[c] >= 0
        xoT[c][:, valid] = xT[:, node_of_slot[c][valid]]

    # weights with [d,h] column order (col d*4+h = original h*64+d)
    W1p = W1.reshape(IN_F, HEADS, HID).transpose(0, 2, 1).reshape(
        IN_F, D1).astype(f16)
    cl1 = np.einsum("khd,hd->kh", W1.reshape(IN_F, HEADS, HID), al1)
    cr1 = np.einsum("khd,hd->kh", W1.reshape(IN_F, HEADS, HID), ar1)
    cw1 = np.concatenate([cl1, cr1], 1).astype(f16)
    # W2: rows reordered to [d1,h1], cols to [d2,h2]
    W2p = W2.reshape(HEADS, HID, HEADS, OUT).transpose(1, 0, 3, 2).reshape(
        D1, D2).astype(f16)
    cl2 = np.einsum("khd,hd->kh", W2.reshape(D1, HEADS, OUT), al2)
    cr2 = np.einsum("khd,hd->kh", W2.reshape(D1, HEADS, OUT), ar2)
    cw2 = np.concatenate([cl2, cr2], 1)
    cw2p = cw2.reshape(HEADS, HID, 8).transpose(1, 0, 2).reshape(
        D1, 8).astype(f16)
    b1p = np.broadcast_to(
        b1.reshape(HEADS, HID).T.reshape(D1).astype(f32), (128, D1)).copy()
    b2m = b2.reshape(HEADS, OUT).mean(0).astype(f32)
    b2m_tile = np.broadcast_to(b2m, (128, OUT)).copy()

    consts = dict(xTp=xTp, W1p=W1p, cw1=cw1, W2p=W2p, cw2p=cw2p,
                  b1p=b1p, b2m_tile=b2m_tile)
    per_core = [dict(xoT=xoT[c], il=il[c], ih=ih[c], dstl=dstl[c])
                for c in range(NCORES)]
    meta = dict(TLs=TLs, THs=THs, node_of_slot=node_of_slot,
                b1z=not np.any(b1), b2z=not np.any(b2))
    return consts, per_core, meta


def _cache_key(meta):
    phases = tuple(os.environ.get(
        "GAT_PHASES", "p0,l1,aga,agb,l2a,l2b").split(","))
    return (meta["TLs"], meta["THs"], phases, meta["b1z"], meta["b2z"],
            os.environ.get("GAT_AGFULL", "1"),
            os.environ.get("GAT_MERGE", "0"),
            os.environ.get("GAT_SP", "0"),
            os.environ.get("GAT_PROBE", ""))


def _build_kernel(TLs, THs, phases=("p0", "l1", "aga", "agb", "l2a", "l2b"),
                  b1z=True, b2z=True):
    import concourse.mybir as mybir
    from concourse import bacc
    from concourse.tile import TileContext
    from concourse.masks import make_identity
    dt = mybir.dt
    AF = mybir.ActivationFunctionType
    OP = mybir.AluOpType
    SL, SHI = sum(TLs), sum(THs)
    ST = SL + SHI
    SP = os.environ.get("GAT_SP", "0") == "1"
    PROBE = os.environ.get("GAT_PROBE", "")  # ""|a (SP=0 probes)|b (SP=1)
    PSP = PROBE == "b"
    TMAX = max(a + b for a, b in zip(TLs, THs))

    nc = bacc.Bacc()

    xTp = nc.dram_tensor("xTp", [IN_F, NP], dt.float16, kind="ExternalInput")
    xoT = nc.dram_tensor("xoT", [IN_F, SH], dt.float16, kind="ExternalInput")
    W1p = nc.dram_tensor("W1p", [IN_F, D1], dt.float16, kind="ExternalInput")
    cw1 = nc.dram_tensor("cw1", [IN_F, 8], dt.float16, kind="ExternalInput")
    W2p = nc.dram_tensor("W2p", [D1, D2], dt.float16, kind="ExternalInput")
    cw2p = nc.dram_tensor("cw2p", [D1, 8], dt.float16, kind="ExternalInput")
    b1p = nc.dram_tensor("b1p", [128, D1], dt.float32, kind="ExternalInput")
    b2m_tile = nc.dram_tensor("b2m_tile", [128, OUT], dt.float32,
                              kind="ExternalInput")
    il = nc.dram_tensor("il", [128, SL * 8], dt.int16, kind="ExternalInput")
    ih = nc.dram_tensor("ih", [128, SHI * 8], dt.int16, kind="ExternalInput")
    dstl = nc.dram_tensor("dstl", [128, ST], dt.float32, kind="ExternalInput")
    out = nc.dram_tensor("out", [SH, OUT], dt.float32, kind="ExternalOutput")

    recs1 = nc.dram_tensor("recs1", [NP, RECW], dt.float16, kind="Internal")
    recs2s = nc.dram_tensor("recs2s", [SH, RECW], dt.float16, kind="Internal")
    recs2fA = nc.dram_tensor("recs2fA", [NA, RECW], dt.float16,
                             kind="Internal", addr_space="Shared")
    recs2fB = nc.dram_tensor("recs2fB", [NB, RECW], dt.float16,
                             kind="Internal", addr_space="Shared")

    _cms = []

    def const_tile(shape, dtype):
        cm = nc.sbuf_tensor(shape, dtype)
        t = cm.__enter__()
        _cms.append(cm)
        return t

    W1sb = const_tile([IN_F, D1], dt.float16)
    cw1sb = const_tile([IN_F, 8], dt.float16)
    W2sb0 = const_tile([128, D2], dt.float16)
    W2sb1 = const_tile([128, D2], dt.float16)
    cw2sb0 = const_tile([128, 8], dt.float16)
    cw2sb1 = const_tile([128, 8], dt.float16)
    b1sb = const_tile([128, D1], dt.float32)
    b2msb = const_tile([128, OUT], dt.float32)
    iotaF = const_tile([128, 128], dt.float16)
    ident16 = const_tile([128, 128], dt.float16)
    ident32 = const_tile([128, 128], dt.float32)
    out_sb = const_tile([128, TILES, OUT], dt.float32)
    er1_sb = const_tile([128, TILES, 4], dt.float16)
    er2_sb = const_tile([128, TILES, 4], dt.float16)
    il_sb = const_tile([128, SL * 8], dt.int16)
    ih_sb = const_tile([128, SHI * 8], dt.int16)
    dl_sb = const_tile([128, ST], dt.float32)
    OHTT = os.environ.get("GAT_OHTT", "0") == "1"
    AGFULL = os.environ.get("GAT_AGFULL", "1") == "1"
    dl16_sb = const_tile([128, ST], dt.float16) if OHTT else None
    Uacc = const_tile([128, TILES, RECU], dt.float32)
    THM = max(THs)
    erB_all = const_tile([128, TILES, 4 * THM], dt.float32)

    # offsets per tile into the ragged arrays
    loff = np.concatenate([[0], np.cumsum(TLs)]).astype(int)
    hoff = np.concatenate([[0], np.cumsum(THs)]).astype(int)
    doff = np.concatenate(
        [[0], np.cumsum(np.asarray(TLs) + np.asarray(THs))]).astype(int)

    # ------------- Phase 0 + layers -------------
    MERGE = os.environ.get("GAT_MERGE", "0") == "1"
    _tc = TileContext(nc)
    tc = _tc.__enter__()
    if True:
        with (tc.tile_pool(name="p0", bufs=3) as p0,
              tc.tile_pool(name="p0ps", bufs=2, space="PSUM") as p0ps,
              tc.tile_pool(name="p0er", bufs=2, space="PSUM") as p0er):
            nc.sync.dma_start(W1sb[:], W1p[:])
            nc.sync.dma_start(cw1sb[:], cw1[:])
            nc.sync.dma_start(W2sb0[:], W2p[0:128, :])
            nc.sync.dma_start(W2sb1[:], W2p[128:256, :])
            nc.sync.dma_start(cw2sb0[:], cw2p[0:128, :])
            nc.sync.dma_start(cw2sb1[:], cw2p[128:256, :])
            nc.sync.dma_start(b1sb[:], b1p[:])
            nc.sync.dma_start(b2msb[:], b2m_tile[:])
            nc.sync.dma_start(il_sb[:], il[:])
            nc.sync.dma_start(ih_sb[:], ih[:])
            nc.sync.dma_start(dl_sb[:], dstl[:])
            if OHTT:
                nc.vector.tensor_copy(dl16_sb[:], dl_sb[:])
            iF32 = p0.tile([128, 128], dt.int32, tag="iF32", bufs=1)
            nc.gpsimd.iota(iF32[:], pattern=[[1, 128]], base=0,
                           channel_multiplier=0)
            nc.vector.tensor_copy(iotaF[:], iF32[:])
            make_identity(nc, ident16[:])
            make_identity(nc, ident32[:])

            if "p0" in phases:
                # er1 for own tiles
                xo = p0.tile([128, SH], dt.float16, tag="xo", bufs=1)
                nc.sync.dma_start(xo[:], xoT[:])
                for t in range(TILES):
                    erps = p0er.tile([128, 4], dt.float32, tag="er1ps",
                                     space="PSUM")
                    nc.tensor.matmul(out=erps[:],
                                     lhsT=xo[:, t * 128:(t + 1) * 128],
                                     rhs=cw1sb[:, 4:8], start=True, stop=True)
                    nc.vector.tensor_copy(er1_sb[:, t, :], erps[:])
                B0 = 8
                for gdx in range(NP // 128 // B0):
                    xt = p0.tile([128, B0 * 128], dt.float16, tag="xt")
                    nc.scalar.dma_start(
                        xt[:], xTp[:, gdx * B0 * 128:(gdx + 1) * B0 * 128])
                    rec = p0.tile([128, B0, 260], dt.float16, tag="rec")
                    eps = p0ps.tile([128, B0 * 8], dt.float32, tag="eps",
                                    bufs=1)
                    for h2 in range(2):
                        zps = p0ps.tile([128, 4 * D1], dt.float32,
                                        tag=f"zps{h2}", bufs=1)
                        for j4 in range(4):
                            j = h2 * 4 + j4
                            nc.tensor.matmul(out=zps[:, j4 * D1:(j4 + 1) * D1],
                                             lhsT=xt[:, j * 128:(j + 1) * 128],
                                             rhs=W1sb[:], start=True,
                                             stop=True)
                            nc.tensor.matmul(out=eps[:, j * 8:(j + 1) * 8],
                                             lhsT=xt[:, j * 128:(j + 1) * 128],
                                             rhs=cw1sb[:], start=True,
                                             stop=True)
                        zv = zps[:].rearrange("p (b d) -> p b d", b=4)
                        nc.scalar.copy(rec[:, h2 * 4:h2 * 4 + 2, 0:D1],
                                       zv[:, 0:2, :])
                        nc.vector.tensor_copy(
                            rec[:, h2 * 4 + 2:h2 * 4 + 4, 0:D1], zv[:, 2:4, :])
                    nc.scalar.copy(
                        rec[:, :, 256:260],
                        eps[:].rearrange("p (b d) -> p b d", b=B0)[:, :, 0:4])
                    nc.sync.dma_start(
                        recs1[gdx * B0 * 128:(gdx + 1) * B0 * 128,
                              0:260].rearrange("(b p) w -> p b w", p=128),
                        rec[:])

    # ---------------- shared edge-chunk machinery ----------------
    def build_ohs(ohs, goff, jn, dl0):
        for j in range(jn):
            nc.vector.tensor_scalar(
                out=ohs[:, goff + j, :], in0=iotaF[:],
                scalar1=dl_sb[:, dl0 + j:dl0 + j + 1],
                scalar2=None, op0=OP.is_equal)

    def er_path(pools, ohs, goff, jn, ert):
        """One-hot transpose + small matmuls -> per-edge er logits (PSUM)."""
        ep, pool_oht, pool_mis = pools
        erps = pool_mis.tile([128, jn * 4], dt.float32, tag="erps",
                             space="PSUM")
        for j0 in range(0, jn, 4):
            j2 = min(4, jn - j0)
            ohT_ps = pool_oht.tile([128, 4, 128], dt.float16, tag="ohT_ps")
            for jj in range(j2):
                nc.tensor.transpose(ohT_ps[:, jj, :],
                                    ohs[:, goff + j0 + jj, :], ident16[:])
            ohT = ep.tile([128, 4, 128], dt.float16, tag="ohT")
            if (j0 // 4) % 4 == 3:
                nc.vector.tensor_copy(ohT[:, 0:j2, :], ohT_ps[:, 0:j2, :])
            else:
                nc.scalar.copy(ohT[:, 0:j2, :], ohT_ps[:, 0:j2, :])
            for jj in range(j2):
                nc.tensor.matmul(out=erps[:, (j0 + jj) * 4:(j0 + jj + 1) * 4],
                                 lhsT=ohT[:, jj, :], rhs=ert,
                                 start=True, stop=True)
        return erps

    def edge_block(pools, jn, goff, g, ohs, dl0, ert, Ups, start, stop,
                   ext_er=None):
        """Process chunks goff..goff+jn of tile: oh, er, lx, exp, alpha, agg.

        g: gather tile [128, jn, RECW] (already gathered, cols 0:260 valid)
        dl0: column offset into dl_sb for chunk 0 of g
        ext_er: optional precomputed per-edge er logits [128, jn, 4] view
        Accumulates into Ups ([128, RECU] PSUM) with given start/stop flags.
        """
        ep, pool_oht, pool_mis = pools
        build_ohs(ohs, goff, jn, dl0)
        if ext_er is None:
            erps = er_path(pools, ohs, goff, jn, ert)
            erv = erps[:].rearrange("p (t f) -> p t f", f=4)
        else:
            erv = ext_er
        lx = ep.tile([128, jn, 4], dt.float32, tag="lx")
        nc.vector.tensor_tensor(
            out=lx[:], in0=g[:, :, 256:260],
            in1=erv, op=OP.add)
        nc.vector.scalar_tensor_tensor(
            out=lx[:], in0=lx[:], scalar=NEG, in1=lx[:],
            op0=OP.mult, op1=OP.max)
        nc.scalar.activation(g[:, :, 260:264], lx[:], AF.Exp)
        nc.vector.tensor_tensor(
            out=g[:, :, 0:D1].rearrange("p t (d h) -> p t d h", h=HEADS),
            in0=g[:, :, 0:D1].rearrange("p t (d h) -> p t d h", h=HEADS),
            in1=g[:, :, 260:264].unsqueeze(2).broadcast_to(
                [128, jn, HID, HEADS]),
            op=OP.mult)
        for j in range(jn):
            nc.tensor.matmul(out=Ups[:], lhsT=ohs[:, goff + j, :],
                             rhs=g[:, j, 0:RECU],
                             start=(start and j == 0),
                             stop=(stop and j == jn - 1))

    # ------------- Layer 1 + AGs + Layer 2 -------------
    if not MERGE:
        _tc.__exit__(None, None, None)
        _tc = TileContext(nc)
        tc = _tc.__enter__()
    if True:
      with (tc.tile_pool(name="ep", bufs=3) as ep,
            tc.tile_pool(name="ups", bufs=2, space="PSUM") as pool_ups,
            tc.tile_pool(name="oht", bufs=2, space="PSUM") as pool_oht,
            tc.tile_pool(name="mis", bufs=1, space="PSUM") as pool_mis):
        pools = (ep, pool_oht, pool_mis)

        if "l1" in phases:
            for t in range(TILES):
                TL, TH = TLs[t], THs[t]
                T = TL + TH
                gf = ep.tile([128, TMAX, RECW], dt.float16, tag="g", bufs=4)
                g = gf[:, 0:T, :]
                if TL:
                    nc.gpsimd.dma_gather(
                        out_ap=g[:, 0:TL, :], in_ap=recs1[0:NA, :],
                        idxs_ap=il_sb[:, loff[t] * 8:(loff[t] + TL) * 8],
                        num_idxs=TL * 128, num_idxs_reg=TL * 128,
                        elem_size=RECW, single_packet=SP)
                if TH:
                    nc.gpsimd.dma_gather(
                        out_ap=g[:, TL:T, :], in_ap=recs1[NA:NP, :],
                        idxs_ap=ih_sb[:, hoff[t] * 8:(hoff[t] + TH) * 8],
                        num_idxs=TH * 128, num_idxs_reg=TH * 128,
                        elem_size=RECW, single_packet=SP)
                if PROBE:
                    pz = ep.tile([128, TMAX, 256], dt.float16, tag="pz",
                                 bufs=1)
                    nc.gpsimd.dma_gather(
                        out_ap=pz[:, 0:TL, :], in_ap=recs1[0:NA, 0:256],
                        idxs_ap=il_sb[:, loff[t] * 8:(loff[t] + TL) * 8],
                        num_idxs=TL * 128, num_idxs_reg=TL * 128,
                        elem_size=256, elem_step=RECW, single_packet=PSP)
                    nc.gpsimd.dma_gather(
                        out_ap=pz[:, TL:T, :], in_ap=recs1[NA:NP, 0:256],
                        idxs_ap=ih_sb[:, hoff[t] * 8:(hoff[t] + TH) * 8],
                        num_idxs=TH * 128, num_idxs_reg=TH * 128,
                        elem_size=256, elem_step=RECW, single_packet=PSP)
                ohsf = ep.tile([128, TMAX, 128], dt.float16, tag="ohs")
                ohs = ohsf[:, 0:T, :]
                Ups = pool_ups.tile([128, RECU], dt.float32, tag="Ups",
                                    space="PSUM")
                edge_block(pools, T, 0, g, ohs, doff[t], er1_sb[:, t, :],
                           Ups, True, True)
                # epilogue: softmax-normalize, bias, ELU, layer-2 records
                s = ep.tile([128, 4], dt.float32, tag="s")
                nc.vector.tensor_scalar_max(s[:], Ups[:, 260:264], 1e-30)
                rs = ep.tile([128, 4], dt.float32, tag="rs")
                nc.vector.reciprocal(rs[:], s[:])
                Uv = Ups[:, 0:D1].rearrange("p (d h) -> p d h", h=HEADS)
                if b1z:
                    rsn = ep.tile([128, 4], dt.float32, tag="rsn")
                    nc.vector.tensor_scalar(out=rsn[:], in0=rs[:],
                                            scalar1=-1.0, scalar2=None,
                                            op0=OP.mult)
                    rn = ep.tile([128, D1], dt.float32, tag="rn")
                    rnv = rn[:].rearrange("p (d h) -> p d h", h=HEADS)
                    rp = ep.tile([128, D1], dt.float32, tag="rp")
                    rpv = rp[:].rearrange("p (d h) -> p d h", h=HEADS)
                    for h in range(HEADS):
                        nc.scalar.activation(rnv[:, :, h], Uv[:, :, h],
                                             AF.Relu, scale=rsn[:, h:h + 1])
                        nc.scalar.activation(rpv[:, :, h], Uv[:, :, h],
                                             AF.Relu, scale=rs[:, h:h + 1])
                    ex = ep.tile([128, D1], dt.float32, tag="ex")
                    nc.scalar.activation(ex[:], rn[:], AF.Exp, scale=-1.0)
                else:
                    x1 = ep.tile([128, D1], dt.float32, tag="x1")
                    nc.vector.tensor_tensor(
                        out=x1[:].rearrange("p (d h) -> p d h", h=HEADS),
                        in0=Uv,
                        in1=rs[:].unsqueeze(1).broadcast_to([128, HID, HEADS]),
                        op=OP.mult)
                    nc.vector.tensor_tensor(x1[:], x1[:], b1sb[:], op=OP.add)
                    rn = ep.tile([128, D1], dt.float32, tag="rn")
                    nc.scalar.activation(rn[:], x1[:], AF.Relu, scale=-1.0)
                    ex = ep.tile([128, D1], dt.float32, tag="ex")
                    nc.scalar.activation(ex[:], rn[:], AF.Exp, scale=-1.0)
                    rp = ep.tile([128, D1], dt.float32, tag="rp")
                    nc.scalar.activation(rp[:], x1[:], AF.Relu)
                hp = ep.tile([128, D1], dt.float32, tag="hp")
                nc.vector.scalar_tensor_tensor(
                    out=hp[:], in0=ex[:], scalar=-1.0, in1=rp[:],
                    op0=OP.add, op1=OP.add)
                hT = ep.tile([128, 256], dt.float16, tag="hT")
                for half in range(2):
                    tp = pool_oht.tile([128, 128], dt.float32, tag="tp",
                                       space="PSUM", bufs=1)
                    nc.tensor.transpose(
                        tp[:], hp[:, half * 128:(half + 1) * 128], ident32[:])
                    nc.scalar.copy(hT[:, half * 128:(half + 1) * 128], tp[:])
                z2ps = pool_mis.tile([128, D2], dt.float32, tag="z2ps",
                                     space="PSUM")
                nc.tensor.matmul(out=z2ps[:], lhsT=hT[:, 0:128], rhs=W2sb0[:],
                                 start=True, stop=False)
                nc.tensor.matmul(out=z2ps[:], lhsT=hT[:, 128:256],
                                 rhs=W2sb1[:], start=False, stop=True)
                e2ps = pool_mis.tile([128, 8], dt.float32, tag="e2ps",
                                     space="PSUM")
                nc.tensor.matmul(out=e2ps[:], lhsT=hT[:, 0:128],
                                 rhs=cw2sb0[:], start=True, stop=False)
                nc.tensor.matmul(out=e2ps[:], lhsT=hT[:, 128:256],
                                 rhs=cw2sb1[:], start=False, stop=True)
                rec2 = ep.tile([128, 260], dt.float16, tag="rec2")
                nc.scalar.copy(rec2[:, 0:D2], z2ps[:])
                nc.scalar.copy(rec2[:, 256:260], e2ps[:, 0:4])
                nc.scalar.copy(er2_sb[:, t, :], e2ps[:, 4:8])
                nc.sync.dma_start(recs2s[t * 128:(t + 1) * 128, 0:260],
                                  rec2[:])
                if t == ATILES - 1 and "aga" in phases:
                    aw = RECW if AGFULL else 260
                    nc.gpsimd.collective_compute(
                        kind="AllGather", op=OP.bypass,
                        replica_groups=[list(range(NCORES))],
                        ins=[recs2s[0:ASL, 0:aw]],
                        outs=[recs2fA[:, 0:aw]])
            if "agb" in phases:
                aw = RECW if AGFULL else 260
                nc.gpsimd.collective_compute(
                    kind="AllGather", op=OP.bypass,
                    replica_groups=[list(range(NCORES))],
                    ins=[recs2s[ASL:SH, 0:aw]],
                    outs=[recs2fB[:, 0:aw]])

        # -------- Layer 2 pass A (region A edges -> Uacc) --------
        if "l2a" in phases:
            for t in range(TILES):
                TL = TLs[t]
                if TL == 0:
                    nc.vector.memset(Uacc[:, t, :], 0.0)
                    continue
                gAf = ep.tile([128, TMAX, RECW], dt.float16, tag="g", bufs=4)
                gA = gAf[:, 0:TL, :]
                nc.gpsimd.dma_gather(
                    out_ap=gA, in_ap=recs2fA[:, :],
                    idxs_ap=il_sb[:, loff[t] * 8:(loff[t] + TL) * 8],
                    num_idxs=TL * 128, num_idxs_reg=TL * 128,
                    elem_size=RECW, single_packet=SP)
                ohsAf = ep.tile([128, TMAX, 128], dt.float16, tag="ohs")
                ohsA = ohsAf[:, 0:TL, :]
                UpsA = pool_ups.tile([128, RECU], dt.float32, tag="Ups",
                                     space="PSUM")
                edge_block(pools, TL, 0, gA, ohsA, doff[t],
                           er2_sb[:, t, :], UpsA, True, True)
                nc.scalar.copy(Uacc[:, t, :], UpsA[:])

        # -------- Layer 2 pass B er precompute (no gather; runs
        # while AG-B is still in flight) --------
        if "l2b" in phases:
            for t in range(TILES):
                TH = THs[t]
                if TH == 0:
                    continue
                ohsPf = ep.tile([128, TMAX, 128], dt.float16, tag="ohs")
                build_ohs(ohsPf[:, 0:TH, :], 0, TH, doff[t] + TLs[t])
                erpsP = er_path(pools, ohsPf[:, 0:TH, :], 0, TH,
                                er2_sb[:, t, :])
                nc.scalar.copy(erB_all[:, t, 0:TH * 4], erpsP[:])

        # -------- Layer 2 pass B (region B edges + epilogue) --------
        if "l2b" in phases:
            for t in range(TILES):
                TH = THs[t]
                Ups = pool_ups.tile([128, RECU], dt.float32, tag="Ups",
                                    space="PSUM")
                nc.tensor.matmul(out=Ups[:], lhsT=ident32[:],
                                 rhs=Uacc[:, t, :], start=True,
                                 stop=(TH == 0))
                if TH:
                    gBf = ep.tile([128, TMAX, RECW], dt.float16, tag="g", bufs=4)
                    gB = gBf[:, 0:TH, :]
                    nc.gpsimd.dma_gather(
                        out_ap=gB, in_ap=recs2fB[:, :],
                        idxs_ap=ih_sb[:, hoff[t] * 8:(hoff[t] + TH) * 8],
                        num_idxs=TH * 128, num_idxs_reg=TH * 128,
                        elem_size=RECW, single_packet=SP)
                    if PROBE:
                        pq = ep.tile([128, TMAX, 128], dt.float16,
                                     tag="pq", bufs=1)
                        nc.gpsimd.dma_gather(
                            out_ap=pq[:, 0:TH, :],
                            in_ap=recs2fB[:, 0:128],
                            idxs_ap=ih_sb[:,
                                          hoff[t] * 8:(hoff[t] + TH) * 8],
                            num_idxs=TH * 128, num_idxs_reg=TH * 128,
                            elem_size=128, elem_step=RECW,
                            single_packet=PSP)
                    ohsBf = ep.tile([128, TMAX, 128], dt.float16, tag="ohs")
                    ohsB = ohsBf[:, 0:TH, :]
                    edge_block(pools, TH, 0, gB, ohsB, doff[t] + TLs[t],
                               er2_sb[:, t, :], Ups, False, True,
                               ext_er=erB_all[:, t, 0:TH * 4].rearrange(
                                   "p (t f) -> p t f", f=4))
                s = ep.tile([128, 4], dt.float32, tag="s2")
                nc.vector.tensor_scalar_max(s[:], Ups[:, 260:264], 1e-30)
                rs = ep.tile([128, 4], dt.float32, tag="rs2")
                nc.vector.reciprocal(rs[:], s[:])
                u = ep.tile([128, D2], dt.float32, tag="u")
                uv = u[:].rearrange("p (d h) -> p d h", h=HEADS)
                Uv2 = Ups[:, 0:D2].rearrange("p (d h) -> p d h", h=HEADS)
                if b2z:
                    for h in range(HEADS):
                        nc.scalar.activation(uv[:, :, h], Uv2[:, :, h],
                                             AF.Copy, scale=rs[:, h:h + 1])
                else:
                    nc.vector.tensor_tensor(out=uv, in0=Uv2,
                                            in1=rs[:].unsqueeze(1).broadcast_to(
                                                [128, OUT, HEADS]),
                                            op=OP.mult)
                red = ep.tile([128, OUT], dt.float32, tag="red")
                nc.vector.tensor_reduce(
                    out=red[:],
                    in_=u[:].rearrange("p (d h) -> p d h", h=HEADS),
                    axis=mybir.AxisListType.X, op=OP.add)
                nc.vector.scalar_tensor_tensor(
                    out=out_sb[:, t, :], in0=red[:], scalar=1.0 / HEADS,
                    in1=b2msb[:], op0=OP.mult, op1=OP.add)
                nc.sync.dma_start(
                    out[t * 128:(t + 1) * 128, :], out_sb[:, t, :])

    _tc.__exit__(None, None, None)
    for cm in reversed(_cms):
        cm.__exit__(None, None, None)
    nc.finalize()
    return nc


_CACHE = {}
_LAST_RESULT = None


def _assemble(results, meta):
    out_full = np.zeros((N, OUT), np.float32)
    node_of_slot = meta["node_of_slot"]
    for c in range(len(results)):
        shard = results[c]["out"]
        valid = node_of_slot[c] >= 0
        out_full[node_of_slot[c][valid]] = shard[valid]
    return out_full


def kernel(x, src, dst, W1, al1, ar1, b1, W2, al2, ar2, b2):
    from concourse.bass_utils import run_bass_kernel_spmd

    args = [np.asarray(a) for a in
            (x, src, dst, W1, al1, ar1, b1, W2, al2, ar2, b2)]
    consts, per_core, meta = _host_prep(*args)
    key = _cache_key(meta)
    if key not in _CACHE:
        _CACHE[key] = _build_kernel(key[0], key[1], phases=key[2],
                                    b1z=key[3], b2z=key[4])
    nc = _CACHE[key]

    in_maps = [{**consts, **per_core[c]} for c in range(NCORES)]
    res = run_bass_kernel_spmd(nc, in_maps, core_ids=list(range(NCORES)))
    global _LAST_RESULT
    _LAST_RESULT = res
    return _assemble(res.results, meta)



# revision 15
# speedup vs baseline: 1.8563x; 1.8563x over previous
"""2-layer GAT on 8 NeuronCores (Trainium2, Bass/Tile) — v2.

Strategy: dst-node sharding with a single global record layout shared by
both layers. Nodes are assigned degree-balanced slots within their owning
core's 49 tiles of 128; the global row order (gperm) is region-major:
region A = slots 0:3200 of each core (rows c*3200+s, 25600 total), region
B = slots 3200:6272 (rows 25600 + c*3072 + (s-3200)). Both layers use the
SAME edge index tensors (ragged per-tile chunk counts, int16 indices with
a lo/hi split at the A/B boundary).

Records are [z(256, stored feature-major d*4+h) | el(4) | ee-scratch(4)]
in rows of stride 384 fp16 (dma_gather needs 256B-aligned elem/stride).
z stored [d,h]-major makes the alpha broadcast-multiply hit the DVE 2x
perf mode (broadcast lands on a middle dim, last dim stays packed).

Layer-2 records are AllGathered in two pieces (full record rows — the
BIR verifier requires contiguous collective patterns): AG-A (region A)
issued after layer-1 tile 24 so it flies under the remaining layer-1
compute; AG-B at layer-1 end. Layer 2 runs in two passes: pass A
aggregates region-A edges into an SBUF accumulator while AG-B is in
flight; pass B adds region-B edges (PSUM preloaded from the accumulator
via an identity matmul) and finishes the epilogue. Epilogues use
ACT-engine per-head scale pointers (1/s softmax normalization fused into
Relu/Copy) when the biases are zero, keeping the vector engine off the
critical path.
"""
import os
import numpy as np

N = 50000
E = 800000
IN_F, HID, OUT, HEADS = 128, 64, 64, 4
D1 = HEADS * HID   # 256
D2 = HEADS * OUT   # 256
NEG = 0.2
NCORES = 8
SHN = N // NCORES          # 6250 dst nodes per core
TILES = 49
SH = TILES * 128           # 6272 slots per core
NP = NCORES * SH           # 50176 rows
ATILES = 17                # region A tiles per core (B capped at 32768 rows for int16)
ASL = ATILES * 128         # 3200 region-A slots per core
BSL = SH - ASL             # 3072 region-B slots
NA = NCORES * ASL          # 25600 region-A rows (lo/hi split)
NB = NCORES * BSL          # 24576 region-B rows
RECW = 384                 # DRAM record stride (fp16 cols); z 0:256, el 256:260
RECU = 264                 # used cols (260 written; 260:264 SBUF ee scratch)


def _host_prep(x, src, dst, W1, al1, ar1, b1, W2, al2, ar2, b2):
    """Pure-numpy preprocessing: slots, global perm, ragged edge chunking."""
    f32, f16 = np.float32, np.float16
    deg = np.bincount(dst, minlength=N)

    # per-core tile assignment (degree balanced round robin)
    slot_of = np.full(N, -1, np.int64)
    node_of_slot = np.full((NCORES, SH), -1, np.int64)
    for c in range(NCORES):
        nodes = np.arange(c * SHN, (c + 1) * SHN)
        order = nodes[np.argsort(-deg[nodes], kind="stable")]
        i = np.arange(order.size)
        s = (i % TILES) * 128 + i // TILES
        slot_of[order] = s
        node_of_slot[c, s] = order

    # global row (both layers): region-major
    cn = np.arange(N) // SHN
    gperm = np.where(slot_of < ASL,
                     cn * ASL + slot_of,
                     NA + cn * BSL + (slot_of - ASL))
    # node stored at row gperm[n]; xTp columns in that order
    node_of_row = np.zeros(NP, np.int64)   # pad rows read node 0 (unused)
    node_of_row[gperm] = np.arange(N)

    ecore = dst // SHN
    etile = slot_of[dst] // 128
    edstl = slot_of[dst] % 128
    erow = gperm[src]

    per = {}
    for c in range(NCORES):
        sel = np.flatnonzero(ecore == c)
        pt = etile[sel]
        for t in range(TILES):
            m = sel[pt == t]
            per[(c, t)] = (erow[m], edstl[m])

    TLs, THs = [], []
    for t in range(TILES):
        nlo = max(int((per[(c, t)][0] < NA).sum()) for c in range(NCORES))
        nhi = max(int((per[(c, t)][0] >= NA).sum()) for c in range(NCORES))
        TLs.append(-(-nlo // 128))
        THs.append(-(-nhi // 128))
    TLs, THs = tuple(TLs), tuple(THs)
    SL, SHI = sum(TLs), sum(THs)

    def pack_idx(vals, TT):
        padded = np.zeros(TT * 128, np.int16)
        padded[:vals.size] = vals.astype(np.int16)
        ii = np.arange(TT * 128)
        w = np.zeros((16, TT * 8), np.int16)
        w[ii % 16, ii // 16] = padded
        return np.tile(w, (8, 1))

    il = np.zeros((NCORES, 128, SL * 8), np.int16)
    ih = np.zeros((NCORES, 128, SHI * 8), np.int16)
    dstl = np.full((NCORES, 128, SL + SHI), -1.0, np.float32)
    ol = oh_ = od = 0
    for t in range(TILES):
        TL, TH = TLs[t], THs[t]
        for c in range(NCORES):
            rows, dl = per[(c, t)]
            lo = rows < NA
            lv, hv = rows[lo], rows[~lo] - NA
            dlo, dhi = dl[lo], dl[~lo]
            if TL:
                il[c, :, ol * 8:(ol + TL) * 8] = pack_idx(lv, TL)
            if TH:
                ih[c, :, oh_ * 8:(oh_ + TH) * 8] = pack_idx(hv, TH)
            for vals, TT, off in ((dlo, TL, od), (dhi, TH, od + TL)):
                if TT == 0:
                    continue
                dpad = np.full(TT * 128, -1.0, f32)
                dpad[:vals.size] = vals
                ii = np.arange(TT * 128)
                dstl[c, ii % 128, off + ii // 128] = dpad
        ol += TL
        oh_ += TH
        od += TL + TH

    # x transposed, global row order (same for all cores)
    xT = np.ascontiguousarray(x.T).astype(f16)          # [128, N]
    xTp = xT[:, node_of_row]                            # [128, NP]
    # per-core own-shard x (slot order) for er computation
    xoT = np.zeros((NCORES, IN_F, SH), f16)
    for c in range(NCORES):
        valid = node_of_slot[c] >= 0
        xoT[c][:, valid] = xT[:, node_of_slot[c][valid]]

    # weights with [d,h] column order (col d*4+h = original h*64+d)
    W1p = W1.reshape(IN_F, HEADS, HID).transpose(0, 2, 1).reshape(
        IN_F, D1).astype(f16)
    cl1 = np.einsum("khd,hd->kh", W1.reshape(IN_F, HEADS, HID), al1)
    cr1 = np.einsum("khd,hd->kh", W1.reshape(IN_F, HEADS, HID), ar1)
    cw1 = np.concatenate([cl1, cr1], 1).astype(f16)
    # W2: rows reordered to [d1,h1], cols to [d2,h2]
    W2p = W2.reshape(HEADS, HID, HEADS, OUT).transpose(1, 0, 3, 2).reshape(
        D1, D2).astype(f16)
    cl2 = np.einsum("khd,hd->kh", W2.reshape(D1, HEADS, OUT), al2)
    cr2 = np.einsum("khd,hd->kh", W2.reshape(D1, HEADS, OUT), ar2)
    cw2 = np.concatenate([cl2, cr2], 1)
    cw2p = cw2.reshape(HEADS, HID, 8).transpose(1, 0, 2).reshape(
        D1, 8).astype(f16)
    b1p = np.broadcast_to(
        b1.reshape(HEADS, HID).T.reshape(D1).astype(f32), (128, D1)).copy()
    b2m = b2.reshape(HEADS, OUT).mean(0).astype(f32)
    b2m_tile = np.broadcast_to(b2m, (128, OUT)).copy()

    consts = dict(xTp=xTp, W1p=W1p, cw1=cw1, W2p=W2p, cw2p=cw2p,
                  b1p=b1p, b2m_tile=b2m_tile)
    per_core = [dict(xoT=xoT[c], il=il[c], ih=ih[c], dstl=dstl[c])
                for c in range(NCORES)]
    meta = dict(TLs=TLs, THs=THs, node_of_slot=node_of_slot,
                b1z=not np.any(b1), b2z=not np.any(b2))
    return consts, per_core, meta


def _cache_key(meta):
    phases = tuple(os.environ.get(
        "GAT_PHASES", "p0,l1,aga,agb,l2a,l2b").split(","))
    return (meta["TLs"], meta["THs"], phases, meta["b1z"], meta["b2z"],
            os.environ.get("GAT_AGFULL", "1"),
            os.environ.get("GAT_MERGE", "0"),
            os.environ.get("GAT_SP", "0"),
            os.environ.get("GAT_PROBE", ""),
            os.environ.get("GAT_PT", "0"))


def _build_kernel(TLs, THs, phases=("p0", "l1", "aga", "agb", "l2a", "l2b"),
                  b1z=True, b2z=True):
    import concourse.mybir as mybir
    from concourse import bacc
    from concourse.tile import TileContext
    from concourse.masks import make_identity
    dt = mybir.dt
    AF = mybir.ActivationFunctionType
    OP = mybir.AluOpType
    SL, SHI = sum(TLs), sum(THs)
    ST = SL + SHI
    SP = os.environ.get("GAT_SP", "0") == "1"
    PROBE = os.environ.get("GAT_PROBE", "")  # ""|a (SP=0 probes)|b (SP=1)
    PSP = PROBE == "b"
    PT = os.environ.get("GAT_PT", "0") == "1"  # prepare_only + trigger_dma
    TMAX = max(a + b for a, b in zip(TLs, THs))

    nc = bacc.Bacc()

    xTp = nc.dram_tensor("xTp", [IN_F, NP], dt.float16, kind="ExternalInput")
    xoT = nc.dram_tensor("xoT", [IN_F, SH], dt.float16, kind="ExternalInput")
    W1p = nc.dram_tensor("W1p", [IN_F, D1], dt.float16, kind="ExternalInput")
    cw1 = nc.dram_tensor("cw1", [IN_F, 8], dt.float16, kind="ExternalInput")
    W2p = nc.dram_tensor("W2p", [D1, D2], dt.float16, kind="ExternalInput")
    cw2p = nc.dram_tensor("cw2p", [D1, 8], dt.float16, kind="ExternalInput")
    b1p = nc.dram_tensor("b1p", [128, D1], dt.float32, kind="ExternalInput")
    b2m_tile = nc.dram_tensor("b2m_tile", [128, OUT], dt.float32,
                              kind="ExternalInput")
    il = nc.dram_tensor("il", [128, SL * 8], dt.int16, kind="ExternalInput")
    ih = nc.dram_tensor("ih", [128, SHI * 8], dt.int16, kind="ExternalInput")
    dstl = nc.dram_tensor("dstl", [128, ST], dt.float32, kind="ExternalInput")
    out = nc.dram_tensor("out", [SH, OUT], dt.float32, kind="ExternalOutput")

    recs1 = nc.dram_tensor("recs1", [NP, RECW], dt.float16, kind="Internal")
    recs2s = nc.dram_tensor("recs2s", [SH, RECW], dt.float16, kind="Internal")
    recs2fA = nc.dram_tensor("recs2fA", [NA, RECW], dt.float16,
                             kind="Internal", addr_space="Shared")
    recs2fB = nc.dram_tensor("recs2fB", [NB, RECW], dt.float16,
                             kind="Internal", addr_space="Shared")

    _cms = []

    def const_tile(shape, dtype):
        cm = nc.sbuf_tensor(shape, dtype)
        t = cm.__enter__()
        _cms.append(cm)
        return t

    W1sb = const_tile([IN_F, D1], dt.float16)
    cw1sb = const_tile([IN_F, 8], dt.float16)
    W2sb0 = const_tile([128, D2], dt.float16)
    W2sb1 = const_tile([128, D2], dt.float16)
    cw2sb0 = const_tile([128, 8], dt.float16)
    cw2sb1 = const_tile([128, 8], dt.float16)
    b1sb = const_tile([128, D1], dt.float32)
    b2msb = const_tile([128, OUT], dt.float32)
    iotaF = const_tile([128, 128], dt.float16)
    ident16 = const_tile([128, 128], dt.float16)
    ident32 = const_tile([128, 128], dt.float32)
    out_sb = const_tile([128, TILES, OUT], dt.float32)
    er1_sb = const_tile([128, TILES, 4], dt.float16)
    er2_sb = const_tile([128, TILES, 4], dt.float16)
    il_sb = const_tile([128, SL * 8], dt.int16)
    ih_sb = const_tile([128, SHI * 8], dt.int16)
    dl_sb = const_tile([128, ST], dt.float32)
    OHTT = os.environ.get("GAT_OHTT", "0") == "1"
    AGFULL = os.environ.get("GAT_AGFULL", "1") == "1"
    dl16_sb = const_tile([128, ST], dt.float16) if OHTT else None
    Uacc = const_tile([128, TILES, RECU], dt.float32)
    THM = max(THs)
    erB_all = const_tile([128, TILES, 4 * THM], dt.float32)

    # offsets per tile into the ragged arrays
    loff = np.concatenate([[0], np.cumsum(TLs)]).astype(int)
    hoff = np.concatenate([[0], np.cumsum(THs)]).astype(int)
    doff = np.concatenate(
        [[0], np.cumsum(np.asarray(TLs) + np.asarray(THs))]).astype(int)

    # ------------- Phase 0 + layers -------------
    MERGE = os.environ.get("GAT_MERGE", "0") == "1"
    _tc = TileContext(nc)
    tc = _tc.__enter__()
    if True:
        with (tc.tile_pool(name="p0", bufs=3) as p0,
              tc.tile_pool(name="p0ps", bufs=2, space="PSUM") as p0ps,
              tc.tile_pool(name="p0er", bufs=2, space="PSUM") as p0er):
            nc.sync.dma_start(W1sb[:], W1p[:])
            nc.sync.dma_start(cw1sb[:], cw1[:])
            nc.sync.dma_start(W2sb0[:], W2p[0:128, :])
            nc.sync.dma_start(W2sb1[:], W2p[128:256, :])
            nc.sync.dma_start(cw2sb0[:], cw2p[0:128, :])
            nc.sync.dma_start(cw2sb1[:], cw2p[128:256, :])
            nc.sync.dma_start(b1sb[:], b1p[:])
            nc.sync.dma_start(b2msb[:], b2m_tile[:])
            nc.sync.dma_start(il_sb[:], il[:])
            nc.sync.dma_start(ih_sb[:], ih[:])
            nc.sync.dma_start(dl_sb[:], dstl[:])
            if OHTT:
                nc.vector.tensor_copy(dl16_sb[:], dl_sb[:])
            iF32 = p0.tile([128, 128], dt.int32, tag="iF32", bufs=1)
            nc.gpsimd.iota(iF32[:], pattern=[[1, 128]], base=0,
                           channel_multiplier=0)
            nc.vector.tensor_copy(iotaF[:], iF32[:])
            make_identity(nc, ident16[:])
            make_identity(nc, ident32[:])

            if "p0" in phases:
                # er1 for own tiles
                xo = p0.tile([128, SH], dt.float16, tag="xo", bufs=1)
                nc.sync.dma_start(xo[:], xoT[:])
                for t in range(TILES):
                    erps = p0er.tile([128, 4], dt.float32, tag="er1ps",
                                     space="PSUM")
                    nc.tensor.matmul(out=erps[:],
                                     lhsT=xo[:, t * 128:(t + 1) * 128],
                                     rhs=cw1sb[:, 4:8], start=True, stop=True)
                    nc.vector.tensor_copy(er1_sb[:, t, :], erps[:])
                B0 = 8
                for gdx in range(NP // 128 // B0):
                    xt = p0.tile([128, B0 * 128], dt.float16, tag="xt")
                    nc.scalar.dma_start(
                        xt[:], xTp[:, gdx * B0 * 128:(gdx + 1) * B0 * 128])
                    rec = p0.tile([128, B0, 260], dt.float16, tag="rec")
                    eps = p0ps.tile([128, B0 * 8], dt.float32, tag="eps",
                                    bufs=1)
                    for h2 in range(2):
                        zps = p0ps.tile([128, 4 * D1], dt.float32,
                                        tag=f"zps{h2}", bufs=1)
                        for j4 in range(4):
                            j = h2 * 4 + j4
                            nc.tensor.matmul(out=zps[:, j4 * D1:(j4 + 1) * D1],
                                             lhsT=xt[:, j * 128:(j + 1) * 128],
                                             rhs=W1sb[:], start=True,
                                             stop=True)
                            nc.tensor.matmul(out=eps[:, j * 8:(j + 1) * 8],
                                             lhsT=xt[:, j * 128:(j + 1) * 128],
                                             rhs=cw1sb[:], start=True,
                                             stop=True)
                        zv = zps[:].rearrange("p (b d) -> p b d", b=4)
                        nc.scalar.copy(rec[:, h2 * 4:h2 * 4 + 2, 0:D1],
                                       zv[:, 0:2, :])
                        nc.vector.tensor_copy(
                            rec[:, h2 * 4 + 2:h2 * 4 + 4, 0:D1], zv[:, 2:4, :])
                    nc.scalar.copy(
                        rec[:, :, 256:260],
                        eps[:].rearrange("p (b d) -> p b d", b=B0)[:, :, 0:4])
                    nc.sync.dma_start(
                        recs1[gdx * B0 * 128:(gdx + 1) * B0 * 128,
                              0:260].rearrange("(b p) w -> p b w", p=128),
                        rec[:])

    # ---------------- shared edge-chunk machinery ----------------
    def build_ohs(ohs, goff, jn, dl0):
        for j in range(jn):
            nc.vector.tensor_scalar(
                out=ohs[:, goff + j, :], in0=iotaF[:],
                scalar1=dl_sb[:, dl0 + j:dl0 + j + 1],
                scalar2=None, op0=OP.is_equal)

    def er_path(pools, ohs, goff, jn, ert):
        """One-hot transpose + small matmuls -> per-edge er logits (PSUM)."""
        ep, pool_oht, pool_mis = pools
        erps = pool_mis.tile([128, jn * 4], dt.float32, tag="erps",
                             space="PSUM")
        for j0 in range(0, jn, 4):
            j2 = min(4, jn - j0)
            ohT_ps = pool_oht.tile([128, 4, 128], dt.float16, tag="ohT_ps")
            for jj in range(j2):
                nc.tensor.transpose(ohT_ps[:, jj, :],
                                    ohs[:, goff + j0 + jj, :], ident16[:])
            ohT = ep.tile([128, 4, 128], dt.float16, tag="ohT")
            if (j0 // 4) % 4 == 3:
                nc.vector.tensor_copy(ohT[:, 0:j2, :], ohT_ps[:, 0:j2, :])
            else:
                nc.scalar.copy(ohT[:, 0:j2, :], ohT_ps[:, 0:j2, :])
            for jj in range(j2):
                nc.tensor.matmul(out=erps[:, (j0 + jj) * 4:(j0 + jj + 1) * 4],
                                 lhsT=ohT[:, jj, :], rhs=ert,
                                 start=True, stop=True)
        return erps

    def edge_block(pools, jn, goff, g, ohs, dl0, ert, Ups, start, stop,
                   ext_er=None):
        """Process chunks goff..goff+jn of tile: oh, er, lx, exp, alpha, agg.

        g: gather tile [128, jn, RECW] (already gathered, cols 0:260 valid)
        dl0: column offset into dl_sb for chunk 0 of g
        ext_er: optional precomputed per-edge er logits [128, jn, 4] view
        Accumulates into Ups ([128, RECU] PSUM) with given start/stop flags.
        """
        ep, pool_oht, pool_mis = pools
        build_ohs(ohs, goff, jn, dl0)
        if ext_er is None:
            erps = er_path(pools, ohs, goff, jn, ert)
            erv = erps[:].rearrange("p (t f) -> p t f", f=4)
        else:
            erv = ext_er
        lx = ep.tile([128, jn, 4], dt.float32, tag="lx")
        nc.vector.tensor_tensor(
            out=lx[:], in0=g[:, :, 256:260],
            in1=erv, op=OP.add)
        nc.vector.scalar_tensor_tensor(
            out=lx[:], in0=lx[:], scalar=NEG, in1=lx[:],
            op0=OP.mult, op1=OP.max)
        nc.scalar.activation(g[:, :, 260:264], lx[:], AF.Exp)
        nc.vector.tensor_tensor(
            out=g[:, :, 0:D1].rearrange("p t (d h) -> p t d h", h=HEADS),
            in0=g[:, :, 0:D1].rearrange("p t (d h) -> p t d h", h=HEADS),
            in1=g[:, :, 260:264].unsqueeze(2).broadcast_to(
                [128, jn, HID, HEADS]),
            op=OP.mult)
        for j in range(jn):
            nc.tensor.matmul(out=Ups[:], lhsT=ohs[:, goff + j, :],
                             rhs=g[:, j, 0:RECU],
                             start=(start and j == 0),
                             stop=(stop and j == jn - 1))

    # ------------- Layer 1 + AGs + Layer 2 -------------
    if not MERGE:
        _tc.__exit__(None, None, None)
        _tc = TileContext(nc)
        tc = _tc.__enter__()
    if True:
      with (tc.tile_pool(name="ep", bufs=3) as ep,
            tc.tile_pool(name="ups", bufs=2, space="PSUM") as pool_ups,
            tc.tile_pool(name="oht", bufs=2, space="PSUM") as pool_oht,
            tc.tile_pool(name="mis", bufs=1, space="PSUM") as pool_mis):
        pools = (ep, pool_oht, pool_mis)
        gsem = nc.alloc_semaphore("gsem") if PT else None

        def do_gather(out_ap, in_ap, idxs_ap, n):
            kw = dict(out_ap=out_ap, in_ap=in_ap, idxs_ap=idxs_ap,
                      num_idxs=n, num_idxs_reg=n, elem_size=RECW,
                      single_packet=SP)
            if PT:
                nc.gpsimd.dma_gather(prepare_only=True, sem=gsem, **kw)
            else:
                nc.gpsimd.dma_gather(**kw)

        def fire():
            if PT:
                nc.gpsimd.trigger_dma(count=None)

        if "l1" in phases:
            for t in range(TILES):
                TL, TH = TLs[t], THs[t]
                T = TL + TH
                gf = ep.tile([128, TMAX, RECW], dt.float16, tag="g", bufs=4)
                g = gf[:, 0:T, :]
                if TL:
                    do_gather(g[:, 0:TL, :], recs1[0:NA, :],
                              il_sb[:, loff[t] * 8:(loff[t] + TL) * 8],
                              TL * 128)
                if TH:
                    do_gather(g[:, TL:T, :], recs1[NA:NP, :],
                              ih_sb[:, hoff[t] * 8:(hoff[t] + TH) * 8],
                              TH * 128)
                fire()
                if PROBE:
                    pz = ep.tile([128, TMAX, 256], dt.float16, tag="pz",
                                 bufs=1)
                    nc.gpsimd.dma_gather(
                        out_ap=pz[:, 0:TL, :], in_ap=recs1[0:NA, 0:256],
                        idxs_ap=il_sb[:, loff[t] * 8:(loff[t] + TL) * 8],
                        num_idxs=TL * 128, num_idxs_reg=TL * 128,
                        elem_size=256, elem_step=RECW, single_packet=PSP)
                    nc.gpsimd.dma_gather(
                        out_ap=pz[:, TL:T, :], in_ap=recs1[NA:NP, 0:256],
                        idxs_ap=ih_sb[:, hoff[t] * 8:(hoff[t] + TH) * 8],
                        num_idxs=TH * 128, num_idxs_reg=TH * 128,
                        elem_size=256, elem_step=RECW, single_packet=PSP)
                ohsf = ep.tile([128, TMAX, 128], dt.float16, tag="ohs")
                ohs = ohsf[:, 0:T, :]
                Ups = pool_ups.tile([128, RECU], dt.float32, tag="Ups",
                                    space="PSUM")
                edge_block(pools, T, 0, g, ohs, doff[t], er1_sb[:, t, :],
                           Ups, True, True)
                # epilogue: softmax-normalize, bias, ELU, layer-2 records
                s = ep.tile([128, 4], dt.float32, tag="s")
                nc.vector.tensor_scalar_max(s[:], Ups[:, 260:264], 1e-30)
                rs = ep.tile([128, 4], dt.float32, tag="rs")
                nc.vector.reciprocal(rs[:], s[:])
                Uv = Ups[:, 0:D1].rearrange("p (d h) -> p d h", h=HEADS)
                if b1z:
                    rsn = ep.tile([128, 4], dt.float32, tag="rsn")
                    nc.vector.tensor_scalar(out=rsn[:], in0=rs[:],
                                            scalar1=-1.0, scalar2=None,
                                            op0=OP.mult)
                    rn = ep.tile([128, D1], dt.float32, tag="rn")
                    rnv = rn[:].rearrange("p (d h) -> p d h", h=HEADS)
                    rp = ep.tile([128, D1], dt.float32, tag="rp")
                    rpv = rp[:].rearrange("p (d h) -> p d h", h=HEADS)
                    for h in range(HEADS):
                        nc.scalar.activation(rnv[:, :, h], Uv[:, :, h],
                                             AF.Relu, scale=rsn[:, h:h + 1])
                        nc.scalar.activation(rpv[:, :, h], Uv[:, :, h],
                                             AF.Relu, scale=rs[:, h:h + 1])
                    ex = ep.tile([128, D1], dt.float32, tag="ex")
                    nc.scalar.activation(ex[:], rn[:], AF.Exp, scale=-1.0)
                else:
                    x1 = ep.tile([128, D1], dt.float32, tag="x1")
                    nc.vector.tensor_tensor(
                        out=x1[:].rearrange("p (d h) -> p d h", h=HEADS),
                        in0=Uv,
                        in1=rs[:].unsqueeze(1).broadcast_to([128, HID, HEADS]),
                        op=OP.mult)
                    nc.vector.tensor_tensor(x1[:], x1[:], b1sb[:], op=OP.add)
                    rn = ep.tile([128, D1], dt.float32, tag="rn")
                    nc.scalar.activation(rn[:], x1[:], AF.Relu, scale=-1.0)
                    ex = ep.tile([128, D1], dt.float32, tag="ex")
                    nc.scalar.activation(ex[:], rn[:], AF.Exp, scale=-1.0)
                    rp = ep.tile([128, D1], dt.float32, tag="rp")
                    nc.scalar.activation(rp[:], x1[:], AF.Relu)
                hp = ep.tile([128, D1], dt.float32, tag="hp")
                nc.vector.scalar_tensor_tensor(
                    out=hp[:], in0=ex[:], scalar=-1.0, in1=rp[:],
                    op0=OP.add, op1=OP.add)
                hT = ep.tile([128, 256], dt.float16, tag="hT")
                for half in range(2):
                    tp = pool_oht.tile([128, 128], dt.float32, tag="tp",
                                       space="PSUM", bufs=1)
                    nc.tensor.transpose(
                        tp[:], hp[:, half * 128:(half + 1) * 128], ident32[:])
                    nc.scalar.copy(hT[:, half * 128:(half + 1) * 128], tp[:])
                z2ps = pool_mis.tile([128, D2], dt.float32, tag="z2ps",
                                     space="PSUM")
                nc.tensor.matmul(out=z2ps[:], lhsT=hT[:, 0:128], rhs=W2sb0[:],
                                 start=True, stop=False)
                nc.tensor.matmul(out=z2ps[:], lhsT=hT[:, 128:256],
                                 rhs=W2sb1[:], start=False, stop=True)
                e2ps = pool_mis.tile([128, 8], dt.float32, tag="e2ps",
                                     space="PSUM")
                nc.tensor.matmul(out=e2ps[:], lhsT=hT[:, 0:128],
                                 rhs=cw2sb0[:], start=True, stop=False)
                nc.tensor.matmul(out=e2ps[:], lhsT=hT[:, 128:256],
                                 rhs=cw2sb1[:], start=False, stop=True)
                rec2 = ep.tile([128, 260], dt.float16, tag="rec2")
                nc.scalar.copy(rec2[:, 0:D2], z2ps[:])
                nc.scalar.copy(rec2[:, 256:260], e2ps[:, 0:4])
                nc.scalar.copy(er2_sb[:, t, :], e2ps[:, 4:8])
                nc.sync.dma_start(recs2s[t * 128:(t + 1) * 128, 0:260],
                                  rec2[:])
                if t == ATILES - 1 and "aga" in phases:
                    aw = RECW if AGFULL else 260
                    nc.gpsimd.collective_compute(
                        kind="AllGather", op=OP.bypass,
                        replica_groups=[list(range(NCORES))],
                        ins=[recs2s[0:ASL, 0:aw]],
                        outs=[recs2fA[:, 0:aw]])
            if "agb" in phases:
                aw = RECW if AGFULL else 260
                nc.gpsimd.collective_compute(
                    kind="AllGather", op=OP.bypass,
                    replica_groups=[list(range(NCORES))],
                    ins=[recs2s[ASL:SH, 0:aw]],
                    outs=[recs2fB[:, 0:aw]])

        # -------- Layer 2 pass A (region A edges -> Uacc) --------
        if "l2a" in phases:
            for t in range(TILES):
                TL = TLs[t]
                if TL == 0:
                    nc.vector.memset(Uacc[:, t, :], 0.0)
                    continue
                gAf = ep.tile([128, TMAX, RECW], dt.float16, tag="g", bufs=4)
                gA = gAf[:, 0:TL, :]
                do_gather(gA, recs2fA[:, :],
                          il_sb[:, loff[t] * 8:(loff[t] + TL) * 8],
                          TL * 128)
                fire()
                ohsAf = ep.tile([128, TMAX, 128], dt.float16, tag="ohs")
                ohsA = ohsAf[:, 0:TL, :]
                UpsA = pool_ups.tile([128, RECU], dt.float32, tag="Ups",
                                     space="PSUM")
                edge_block(pools, TL, 0, gA, ohsA, doff[t],
                           er2_sb[:, t, :], UpsA, True, True)
                nc.scalar.copy(Uacc[:, t, :], UpsA[:])

        # -------- Layer 2 pass B er precompute (no gather; runs
        # while AG-B is still in flight) --------
        if "l2b" in phases:
            for t in range(TILES):
                TH = THs[t]
                if TH == 0:
                    continue
                ohsPf = ep.tile([128, TMAX, 128], dt.float16, tag="ohs")
                build_ohs(ohsPf[:, 0:TH, :], 0, TH, doff[t] + TLs[t])
                erpsP = er_path(pools, ohsPf[:, 0:TH, :], 0, TH,
                                er2_sb[:, t, :])
                nc.scalar.copy(erB_all[:, t, 0:TH * 4], erpsP[:])

        # -------- Layer 2 pass B (region B edges + epilogue) --------
        if "l2b" in phases:
            for t in range(TILES):
                TH = THs[t]
                Ups = pool_ups.tile([128, RECU], dt.float32, tag="Ups",
                                    space="PSUM")
                nc.tensor.matmul(out=Ups[:], lhsT=ident32[:],
                                 rhs=Uacc[:, t, :], start=True,
                                 stop=(TH == 0))
                if TH:
                    gBf = ep.tile([128, TMAX, RECW], dt.float16, tag="g", bufs=4)
                    gB = gBf[:, 0:TH, :]
                    do_gather(gB, recs2fB[:, :],
                              ih_sb[:, hoff[t] * 8:(hoff[t] + TH) * 8],
                              TH * 128)
                    fire()
                    if PROBE:
                        pq = ep.tile([128, TMAX, 128], dt.float16,
                                     tag="pq", bufs=1)
                        nc.gpsimd.dma_gather(
                            out_ap=pq[:, 0:TH, :],
                            in_ap=recs2fB[:, 0:128],
                            idxs_ap=ih_sb[:,
                                          hoff[t] * 8:(hoff[t] + TH) * 8],
                            num_idxs=TH * 128, num_idxs_reg=TH * 128,
                            elem_size=128, elem_step=RECW,
                            single_packet=PSP)
                    ohsBf = ep.tile([128, TMAX, 128], dt.float16, tag="ohs")
                    ohsB = ohsBf[:, 0:TH, :]
                    edge_block(pools, TH, 0, gB, ohsB, doff[t] + TLs[t],
                               er2_sb[:, t, :], Ups, False, True,
                               ext_er=erB_all[:, t, 0:TH * 4].rearrange(
                                   "p (t f) -> p t f", f=4))
                s = ep.tile([128, 4], dt.float32, tag="s2")
                nc.vector.tensor_scalar_max(s[:], Ups[:, 260:264], 1e-30)
                rs = ep.tile([128, 4], dt.float32, tag="rs2")
                nc.vector.reciprocal(rs[:], s[:])
                u = ep.tile([128, D2], dt.float32, tag="u")
                uv = u[:].rearrange("p (d h) -> p d h", h=HEADS)
                Uv2 = Ups[:, 0:D2].rearrange("p (d h) -> p d h", h=HEADS)
                if b2z:
                    for h in range(HEADS):
                        nc.scalar.activation(uv[:, :, h], Uv2[:, :, h],
                                             AF.Copy, scale=rs[:, h:h + 1])
                else:
                    nc.vector.tensor_tensor(out=uv, in0=Uv2,
                                            in1=rs[:].unsqueeze(1).broadcast_to(
                                                [128, OUT, HEADS]),
                                            op=OP.mult)
                red = ep.tile([128, OUT], dt.float32, tag="red")
                nc.vector.tensor_reduce(
                    out=red[:],
                    in_=u[:].rearrange("p (d h) -> p d h", h=HEADS),
                    axis=mybir.AxisListType.X, op=OP.add)
                nc.vector.scalar_tensor_tensor(
                    out=out_sb[:, t, :], in0=red[:], scalar=1.0 / HEADS,
                    in1=b2msb[:], op0=OP.mult, op1=OP.add)
                nc.sync.dma_start(
                    out[t * 128:(t + 1) * 128, :], out_sb[:, t, :])

    _tc.__exit__(None, None, None)
    for cm in reversed(_cms):
        cm.__exit__(None, None, None)
    nc.finalize()
    return nc


_CACHE = {}
_LAST_RESULT = None


def _assemble(results, meta):
    out_full = np.zeros((N, OUT), np.float32)
    node_of_slot = meta["node_of_slot"]
    for c in range(len(results)):
        shard = results[c]["out"]
        valid = node_of_slot[c] >= 0
        out_full[node_of_slot[c][valid]] = shard[valid]
    return out_full


def kernel(x, src, dst, W1, al1, ar1, b1, W2, al2, ar2, b2):
    from concourse.bass_utils import run_bass_kernel_spmd

    args = [np.asarray(a) for a in
            (x, src, dst, W1, al1, ar1, b1, W2, al2, ar2, b2)]
    consts, per_core, meta = _host_prep(*args)
    key = _cache_key(meta)
    if key not in _CACHE:
        _CACHE[key] = _build_kernel(key[0], key[1], phases=key[2],
                                    b1z=key[3], b2z=key[4])
    nc = _CACHE[key]

    in_maps = [{**consts, **per_core[c]} for c in range(NCORES)]
    res = run_bass_kernel_spmd(nc, in_maps, core_ids=list(range(NCORES)))
    global _LAST_RESULT
    _LAST_RESULT = res
    return _assemble(res.results, meta)

